# revision 1
# baseline (speedup 1.0000x reference)
"""Trainium2 Bass kernel for batch-axis-softmax dot-product attention.

Problem: B=8, S=4096, D=64 fp32.
    scores = einsum('bqd,bkd->bqk', Q, K) / 8
    attn   = softmax(scores, axis=0)          # over the BATCH axis!
    out    = einsum('bqk,bkd->bqd', attn, V)

The batch-axis softmax couples only the 8 batch entries of a fixed (q, k)
position, so sharding over the *query* axis (512 queries per core, K/V
replicated) keeps the softmax fully local to each core.

Per-core pipeline, per k-tile (128 keys x 512 queries, all 8 batches):
  PE : scoresT[k,q] = K_tile @ Q^T   (fp16, fp32 psum; batch pairs packed
       into partition halves -> row-tiled concurrent MMs; each pair's two
       512-wide outputs land in one 2-bank psum tile)
  ACT: E_pair = exp(0.125 * scores_pair)  (one 1024-wide op per pair)
  DVE: Z = sum over the 8 batches (fp16 tree of 5 tensor_adds; GpSimd is
       avoided on purpose — its SBUF-port sharing triples concurrent DVE ops)
  ACT: R = exp(-ln(Z)) = 1/Z, fp16, once per k-tile PAIR at full width
       (ln+exp share the natural_log_exp_and_others table set -> 1 load)
  DVE: W_b = E_b * R  (fp16 2x mode, R pair-broadcast via stride-0 AP)
  PE : outT_b[d,q] += V_tile matmul, accumulated across all 32 k-tiles in
       persistent psum (2 batches per bank via column tiling)
Epilogue: DVE copies psum -> sbuf, DMA to HBM; host reassembles.

Scheduling: a 2-tile-lag software pipeline with AV matmuls drained between
score packs keeps ScalarE (the bottleneck stream, ~97% busy in steady
state) fed. Steady state is ScalarE-bound at ~5.4us per k-tile; HW exec
~196us on 8 cores (vs ~445us for the first working version).
"""

import numpy as np

B = 8
S = 4096
D = 64
NCORES = 8
QBLK = S // NCORES  # 512 queries per core
KT = 128            # keys per k-tile
NKT = S // KT       # 32 k-tiles
NPAIR = B // 2      # batch pairs packed into 128 partitions

# test.py can flip these before calling kernel()
TRACE = False
TRACE_KWARGS = {}
LAST_RESULT = None  # BassKernelResults of the most recent run (for profiling)

_cache = {}


def _build_nc():
    from contextlib import ExitStack

    import concourse.tile as tile
    from concourse import bacc, mybir

    f16 = mybir.dt.float16
    f32 = mybir.dt.float32
    Exp = mybir.ActivationFunctionType.Exp
    Ln = mybir.ActivationFunctionType.Ln

    # Bacc (not raw Bass): its finalize() runs the legalization passes that
    # split multi-wait sync_info into EventSemaphore instructions (TRN2 allows
    # at most one wait per regular instruction).
    #
    # insert_act_table_loads maps each activation func to the first table set
    # containing it, which puts Exp in "exp_and_others" and Ln in
    # "natural_log_exp_and_others" — alternating ACT_TABLE_LOADs every k-tile
    # (~80us of ScalarE). Both funcs live in natural_log_exp_and_others, so
    # restrict Exp/Ln membership to that set: one table load for the whole
    # kernel, hoisted out of the loop.
    class _Bacc(bacc.Bacc):
        def insert_act_table_loads(self):
            from concourse import bass as bass_mod
            from concourse.hw_specs import get_activation_tables

            has_activation = any(
                isinstance(i, mybir.InstActivation)
                for b in self.main_func.blocks
                for i in b.instructions
            )
            if not has_activation:
                return
            combined = "natural_log_exp_and_others"
            tables = []
            for name, fns in get_activation_tables(self.m.arch).items():
                if name != combined:
                    fns = fns - {
                        mybir.ActivationFunctionType.Exp,
                        mybir.ActivationFunctionType.Ln,
                    }
                tables.append((name, fns))
            bass_mod._bass_rust.insert_act_table_loads(self, tables)

    nc = _Bacc()

    # Inputs pre-arranged on host into exact SBUF layouts (fp16):
    #   qt[p, j*512 + q] = Q[2j + p//64, cblk*512 + q, p%64]
    #   kt[p, j*4096 + k] = K[2j + p//64, k, p%64]
    #   vv[p, b*2048 + n*64 + d] = V[b, n*128 + p, d]
    qt_d = nc.dram_tensor("qt", [128, NPAIR * QBLK], f16, kind="ExternalInput")
    kt_d = nc.dram_tensor("kt", [128, NPAIR * S], f16, kind="ExternalInput")
    vv_d = nc.dram_tensor("vv", [128, B * NKT * D], f16, kind="ExternalInput")
    # out[j][(b%2)*64 + d, q] = out_bqd[2j + b%2, q, d]
    out_d = nc.dram_tensor("out", [NPAIR, 128, QBLK], f32, kind="ExternalOutput")

    with tile.TileContext(nc) as tc, ExitStack() as ctx:
        in_p = ctx.enter_context(tc.tile_pool(name="inp", bufs=1))
        e_p = ctx.enter_context(tc.tile_pool(name="e", bufs=17))
        w_p = ctx.enter_context(tc.tile_pool(name="w", bufs=10))
        t_p = ctx.enter_context(tc.tile_pool(name="tree", bufs=3))
        r_p = ctx.enter_context(tc.tile_pool(name="r", bufs=2))
        st_p = ctx.enter_context(tc.tile_pool(name="stage", bufs=1))
        ps_s = ctx.enter_context(tc.tile_pool(name="ps_s", bufs=2, space="PSUM"))
        ps_o = ctx.enter_context(tc.tile_pool(name="ps_o", bufs=1, space="PSUM"))

        # kt/vv are laid out k-tile-major on the host and DMA'd in per-tile
        # chunks interleaved kt/vv, so tile 0's operands land ~7us in and the
        # loop never waits on later chunks.
        qt = in_p.tile([128, NPAIR * QBLK], f16)
        kt = in_p.tile([128, NKT * NPAIR * KT], f16)
        vv = in_p.tile([128, NKT * B * D], f16)
        CH = NPAIR * KT  # 512 columns per k-tile chunk (for both kt and vv)

        def dma_col(dst, src, c0, c1):
            nc.sync.dma_start(out=dst[:, c0:c1], in_=src[:, c0:c1])

        # Issue order: operands of score pack (t=0, j=0) first, then the
        # rest of tile 0, then per-tile chunks so the loop never waits.
        dma_col(qt, qt_d, 0, QBLK)
        dma_col(kt, kt_d, 0, CH)
        for j in range(1, NPAIR):
            dma_col(qt, qt_d, j * QBLK, (j + 1) * QBLK)
        dma_col(vv, vv_d, 0, CH)
        for t in range(1, NKT):
            dma_col(kt, kt_d, t * CH, (t + 1) * CH)
            dma_col(vv, vv_d, t * CH, (t + 1) * CH)

        # Persistent output accumulators: bank j holds batches 2j (parts
        # 0:64) and 2j+1 (parts 64:128), accumulated over all 32 k-tiles.
        oacc = [
            ps_o.tile([128, QBLK], f32, tag=f"oacc{j}", name=f"oacc{j}")
            for j in range(NPAIR)
        ]

        # AV matmuls pending issue; drained between score packs so PE always
        # services the (ACT-feeding) score matmuls promptly instead of
        # running 16-MM AV bursts that starve ScalarE. Interleaving AV MMs
        # of adjacent k-tiles is safe: psum accumulate-adds commute.
        av_pending = []

        def drain_av(n):
            for _ in range(min(n, len(av_pending))):
                av_pending.pop(0)()

        def emit_scores_exp(t):
            # scores + exp, one 2-bank pack per batch pair
            e_packs = []
            for j in range(NPAIR):
                sc = ps_s.tile([128, 2 * QBLK], f32, tag="sc", name=f"sc{t}_{j}")
                for m in range(2):  # m=0 -> b=2j (rows 0:64), m=1 -> b=2j+1
                    rb = m * 64
                    nc.tensor.matmul(
                        out=sc[:, m * QBLK : (m + 1) * QBLK],
                        lhsT=kt[rb : rb + 64, t * CH + j * KT : t * CH + (j + 1) * KT],
                        rhs=qt[rb : rb + 64, j * QBLK : (j + 1) * QBLK],
                        start=True,
                        stop=True,
                        tile_position=(rb, 0),
                    )
                e = e_p.tile([128, 2 * QBLK], f16, tag="e", name=f"e{t}_{j}")
                # E = exp(scores / sqrt(D)); scores*0.125 in [-6, 6] so no
                # max-subtraction is needed and fp16 range is safe.
                nc.scalar.activation(e[:], sc[:], Exp, scale=0.125)
                e_packs.append(e)
                drain_av(2)
            return e_packs

        def emit_tree(t, e_packs):
            # Z = sum_b E_b, all on DVE. GpSimd is deliberately NOT used:
            # any GpSimd tensor op grabs the shared SBUF port and stretches
            # concurrent DVE tensor_tensor ops ~3x (measured 418ns -> 1370ns).
            # Pack-level adds keep the op count at 5 (2 full-width + 3 half).
            t01 = t_p.tile([128, 2 * QBLK], f16, tag="t01", name=f"t01_{t}")
            nc.vector.tensor_add(t01[:], e_packs[0][:], e_packs[1][:])
            t23 = t_p.tile([128, 2 * QBLK], f16, tag="t23", name=f"t23_{t}")
            nc.vector.tensor_add(t23[:], e_packs[2][:], e_packs[3][:])
            u0 = t_p.tile([128, QBLK], f16, tag="u0", name=f"u0_{t}")
            nc.vector.tensor_add(u0[:], t01[:, :QBLK], t01[:, QBLK:])
            u1 = t_p.tile([128, QBLK], f16, tag="u1", name=f"u1_{t}")
            nc.vector.tensor_add(u1[:], t23[:, :QBLK], t23[:, QBLK:])
            # z tiles of consecutive k-tiles share one [128, 1024] pair tile
            # so ln/exp run once per PAIR at full width (less ACT overhead).
            if t % 2 == 0:
                zp = t_p.tile([128, 2 * QBLK], f16, tag="zp", name=f"zp{t}", bufs=3)
                zpairs[t // 2] = zp
            zp = zpairs[t // 2]
            nc.vector.tensor_add(
                zp[:, (t % 2) * QBLK : (t % 2 + 1) * QBLK], u0[:], u1[:]
            )

        def emit_r_pair(tp):
            # R = 1/Z via exp(-ln(Z)) on ScalarE (shared table set), for a
            # pair of k-tiles (2*tp, 2*tp+1) in one full-width op each.
            zp = zpairs.pop(tp)
            lnz = r_p.tile([128, 2 * QBLK], f32, tag="lnz", name=f"lnz{tp}")
            nc.scalar.activation(lnz[:], zp[:], Ln)
            r16 = r_p.tile([128, 2 * QBLK], f16, tag="r16", name=f"r16_{tp}", bufs=3)
            nc.scalar.activation(r16[:], lnz[:], Exp, scale=-1.0)
            return r16

        def emit_mults(t, e_packs, r16, half):
            # W_b = E_b * R, one fp16 2x-mode op per pack with R broadcast
            # over the pair dimension via a stride-0 access pattern.
            r = r16[:, half * QBLK : (half + 1) * QBLK]
            ws = []
            for j in range(NPAIR):
                w = w_p.tile([128, 2 * QBLK], f16, tag="w", name=f"w{t}_{j}")
                nc.vector.tensor_mul(
                    w[:].rearrange("p (a q) -> p a q", a=2),
                    e_packs[j][:].rearrange("p (a q) -> p a q", a=2),
                    r.rearrange("p (a q) -> p a q", a=1).to_broadcast(
                        (128, 2, QBLK)
                    ),
                )
                ws.append(w)
            return ws

        def emit_av(t, ws):
            # outT_b[d,q] += V_b[t]^T-form matmul, queued for interleaved
            # issue (see drain_av). Reverse order so the first-issued AV's
            # wait (on the last mult's DVE tick) covers the others.
            def mk(b):
                j, m = b // 2, b % 2
                rb = m * 64

                def go():
                    nc.tensor.matmul(
                        out=oacc[j][rb : rb + 64, :],
                        lhsT=vv[:, t * CH + b * D : t * CH + (b + 1) * D],
                        rhs=ws[j][:, m * QBLK : (m + 1) * QBLK],
                        start=(t == 0),
                        stop=(t == NKT - 1),
                        tile_position=(0, rb),
                        skip_group_check=True,
                    )

                return go

            for b in reversed(range(B)):
                av_pending.append(mk(b))

        # Software pipeline with a 2-tile lag between the exp/tree front end
        # and the ln/mult/AV back end: every cross-engine input reaching an
        # engine's strict FIFO was produced >= 2 tiles earlier, so waits are
        # pre-satisfied and each engine streams without head-of-line stalls
        # (a 1-tile lag still stalled ACT ~2us per tile on the add-tree).
        # 2-tile-lag software pipeline: the R/mult/AV back end consumes data
        # produced two k-tiles earlier, so its waits are pre-satisfied when
        # they reach each engine's strict FIFO. The back end runs per PAIR of
        # k-tiles (on odd back-tiles) to use full-width ln/exp ops.
        LAG = 2
        zpairs = {}
        state = {}  # t -> e_packs

        def back_end(tp):
            r16 = emit_r_pair(tp)
            for u, half in ((2 * tp, 0), (2 * tp + 1, 1)):
                ws = emit_mults(u, state.pop(u), r16, half)
                emit_av(u, ws)

        for t in range(NKT + LAG):
            tb = t - LAG
            # Back end runs per pair on odd back-tiles; the final pair is
            # pulled one iteration earlier (its z is ready by then). In the
            # last iterations the back end is emitted BEFORE the scores/exps
            # so its R-ops sit ahead of the final exp block in ScalarE's
            # FIFO and the mults/AVs overlap it instead of trailing it.
            do_back = (
                tb >= 0
                and (tb % 2 == 1 or tb == NKT - 2)
                and (tb | 1) // 2 in zpairs
            )
            if t < NKT:
                e_packs = emit_scores_exp(t)
                emit_tree(t, e_packs)
                state[t] = e_packs
            if do_back:
                back_end((tb | 1) // 2)
        drain_av(len(av_pending))

        # One staging tile + one output DMA (4 separate dma_starts each pay
        # ~2us of setup/completion latency in the kernel tail).
        st = st_p.tile([128, NPAIR * QBLK], f32, tag="st")
        for j in range(NPAIR):
            nc.vector.tensor_copy(
                out=st[:, j * QBLK : (j + 1) * QBLK], in_=oacc[j][:]
            )
        nc.sync.dma_start(
            out=out_d[:].rearrange("j p q -> p j q"),
            in_=st[:].rearrange("p (j q) -> p j q", j=NPAIR),
        )

    return nc


def _get_nc():
    if "nc" not in _cache:
        nc = _build_nc()
        if not nc.is_finalized():
            # Runs Bacc.compile() legalization (wait splitting, reg alloc).
            nc.finalize()
        _cache["nc"] = nc
    return _cache["nc"]


def _host_prep(queries, keys, values):
    """Cast to fp16 and pre-arrange into the SBUF layouts (see _build_nc)."""
    k16 = np.asarray(keys, dtype=np.float16)
    v16 = np.asarray(values, dtype=np.float16)
    q16 = np.asarray(queries, dtype=np.float16)

    # kt[(b%2)*64+d, t*512 + (b//2)*128 + kk] = K[b, t*128+kk, d] (k-tile major)
    kt = np.ascontiguousarray(
        k16.reshape(NPAIR, 2, NKT, KT, D)
        .transpose(1, 4, 2, 0, 3)
        .reshape(128, NKT * NPAIR * KT)
    )
    # vv[p, t*512 + b*64 + d] = V[b, t*128+p, d] (k-tile major)
    vv = np.ascontiguousarray(
        v16.reshape(B, NKT, KT, D).transpose(2, 1, 0, 3).reshape(128, NKT * B * D)
    )

    qts = []
    for c in range(NCORES):
        qc = q16[:, c * QBLK : (c + 1) * QBLK, :]  # [8, 512, 64]
        qt = np.ascontiguousarray(
            qc.transpose(0, 2, 1).reshape(NPAIR, 128, QBLK).transpose(1, 0, 2).reshape(128, NPAIR * QBLK)
        )
        qts.append(qt)
    return qts, kt, vv


def kernel(queries, keys, values):
    global LAST_RESULT
    from concourse.bass_utils import run_bass_kernel_spmd

    queries = np.asarray(queries, dtype=np.float32)
    keys = np.asarray(keys, dtype=np.float32)
    values = np.asarray(values, dtype=np.float32)

    nc = _get_nc()
    qts, kt, vv = _host_prep(queries, keys, values)
    in_maps = [{"qt": qts[c], "kt": kt, "vv": vv} for c in range(NCORES)]

    res = run_bass_kernel_spmd(
        nc,
        in_maps,
        list(range(NCORES)),
        trace=TRACE,
        **TRACE_KWARGS,
    )
    LAST_RESULT = res

    out = np.empty((B, S, D), dtype=np.float32)
    for c in range(NCORES):
        o = res.results[c]["out"]  # [4, 128, 512] = [j, (b%2)*64+d, q]
        out[:, c * QBLK : (c + 1) * QBLK, :] = (
            o.reshape(B, D, QBLK).transpose(0, 2, 1)
        )
    return out



# revision 3
# speedup vs baseline: 1.1286x; 1.1286x over previous
"""Trainium2 Bass kernel for batch-axis-softmax dot-product attention.

Problem: B=8, S=4096, D=64 fp32.
    scores = einsum('bqd,bkd->bqk', Q, K) / 8
    attn   = softmax(scores, axis=0)          # over the BATCH axis!
    out    = einsum('bqk,bkd->bqd', attn, V)

The batch-axis softmax couples only the 8 batch entries of a fixed (q, k)
position, so sharding over the *query* axis (512 queries per core, K/V
replicated) keeps the softmax fully local to each core.

Per-core pipeline, per k-tile (128 keys x 512 queries, all 8 batches):
  PE : scoresT[k,q] = K_tile @ Q^T   (fp16, fp32 psum; batch pairs packed
       into partition halves -> row-tiled concurrent MMs; each pair's two
       512-wide outputs land in one 2-bank psum tile)
  ACT: e-quarter = exp(0.125 * scores_pack), all four packs of one k-tile
       written into ONE contiguous [128, 4096] fp16 e-tile
  DVE: two fused tree adds halve 8 batches -> 2 partial sums, then a
       CUSTOM DVE op (ZSUM_RECIP_ANT, registered at build time) computes
       r = 1/(u0+u1) in a single pass: BITWISE_NOT exponent-flip seed +
       one inline Newton step (~2e-3 max rel err, fp16 out). This replaces
       the baseline's ScalarE ln/exp reciprocal AND the last tree level,
       taking ScalarE out of the normalizer entirely.
  DVE: W = E * r  as ONE [128, 8x512] fp16 2x-mode tensor_tensor with r
       broadcast over the 8 batch chunks via a stride-0 access pattern
  PE : outT_b[d,q] += V_tile matmul, accumulated across all 32 k-tiles in
       persistent psum (2 batches per bank via column tiling)
Epilogue: DVE copies psum -> sbuf, DMA to HBM; host reassembles.

Engine budget per k-tile (measured rates): ACT 4x1140ns = 4.56us,
DVE 1224+690+572+2290 = 4.78us, PE ~3.5us -> ACT/DVE balanced at ~150us
for 32 tiles vs the baseline's ScalarE-bound 5.73us/tile (~200us).

The reciprocal groups span 4 k-tiles (2 for the last two groups to cut
the pipeline tail): one ZSUM_RECIP op per group amortizes DVE op setup.
"""

import numpy as np

B = 8
S = 4096
D = 64
NCORES = 8
QBLK = S // NCORES  # 512 queries per core
KT = 128            # keys per k-tile
NKT = S // KT       # 32 k-tiles
NPAIR = B // 2      # batch pairs packed into 128 partitions

# Reciprocal groups: (start_tile, ntiles). 4-tile groups amortize DVE op
# overhead; two 2-tile groups at the end shorten the post-loop tail.
GROUPS = [(0, 4), (4, 4), (8, 4), (12, 4), (16, 4), (20, 4), (24, 4),
          (28, 2), (30, 2)]

# test.py can flip these before calling kernel()
TRACE = False
TRACE_KWARGS = {}
LAST_RESULT = None  # BassKernelResults of the most recent run (for profiling)

_cache = {}


def _register_zsum_recip():
    """Register the fused r = 1/(a+b) custom DVE op (seed + 1 Newton step).

    nc.vector.reciprocal is ~6 cycles/elem and ScalarE ln/exp costs ~1.1us
    per [128,1024]; this runs at 1 elem/cycle/lane in one DVE pass and also
    absorbs the final level of the batch-sum tree. Seed trick (from
    RECIPROCAL_APPROX_FAST): BITWISE_NOT of the fp32 bit pattern flips the
    exponent so x*~x lands in [-4.5,-4]; one Chebyshev scale + one NR pass
    gives ~2e-3 max rel error, plenty under the fp16 output rounding that
    follows. 6 ALU stages of the 8 available.
    """
    from concourse import dve_ops
    from concourse.dve_spec import AluOp, Bin, C0, C1, Spec, Src0, Src1, lower
    from concourse.dve_spec import _has_src1
    from concourse.dve_uop import DveOpSpec

    name = "ZSUM_RECIP_ANT"
    if name in dve_ops._SUB_OPCODE_FOR_NAME:
        return next(op for op in dve_ops.OPS if op.name == name)

    _z = Src0 + Src1
    _nz = Bin(AluOp.BITWISE_NOT, _z, _z)
    _y0 = _nz * C0
    body = _y0 * (C1 - _z * _y0)

    def ref(in0, in1, s0, s1, imm2):
        zz = in0.astype(np.float32) + in1.astype(np.float32)
        nz = (~zz.view(np.int32)).view(np.float32)
        y0 = nz * np.float32(s0)
        return y0 * (np.float32(s1) - zz * y0)

    spec = Spec(body=body, reference=ref)
    row = dve_ops._CUSTOM_DVE_ROW_BASE + len(dve_ops.OPS)
    assert row < 0x20
    shas = {}
    for ver in ("v3", "v4"):
        s = DveOpSpec(name=name, opcode=row, uops=lower(spec, ver=ver),
                      rd1_en=_has_src1(spec))
        shas[ver] = s.sha(ver)
    op = dve_ops.DveOp(name, spec, subdim=False, uops_sha=shas)
    dve_ops.OPS.append(op)
    dve_ops.CUSTOM_DVE_SPECS[name] = spec
    dve_ops._SUB_OPCODE_FOR_NAME[name] = row
    return op


# Chebyshev-minimax seed constants (see dve_ops.RECIP_APPROX_FAST_CONSTS).
_RECIP_C0 = -0.23549792
_RECIP_C1 = 2.0017324


def _build_nc():
    from contextlib import ExitStack

    import concourse.tile as tile
    from concourse import bacc, mybir

    zsum_recip = _register_zsum_recip()

    f16 = mybir.dt.float16
    f32 = mybir.dt.float32
    Exp = mybir.ActivationFunctionType.Exp

    # Bacc (not raw Bass): its finalize() runs the legalization passes that
    # split multi-wait sync_info into EventSemaphore instructions (TRN2 allows
    # at most one wait per regular instruction). Only Exp is used, so the
    # default table-load insertion emits a single hoisted ACT_TABLE_LOAD.
    nc = bacc.Bacc()

    # Inputs pre-arranged on host into exact SBUF layouts (fp16):
    #   qt[p, j*512 + q] = Q[2j + p//64, cblk*512 + q, p%64]
    #   kt[p, j*4096 + k] = K[2j + p//64, k, p%64]
    #   vv[p, b*2048 + n*64 + d] = V[b, n*128 + p, d]
    qt_d = nc.dram_tensor("qt", [128, NPAIR * QBLK], f16, kind="ExternalInput")
    kt_d = nc.dram_tensor("kt", [128, NPAIR * S], f16, kind="ExternalInput")
    vv_d = nc.dram_tensor("vv", [128, B * NKT * D], f16, kind="ExternalInput")
    # out[j][(b%2)*64 + d, q] = out_bqd[2j + b%2, q, d]
    out_d = nc.dram_tensor("out", [NPAIR, 128, QBLK], f32, kind="ExternalOutput")

    with tile.TileContext(nc) as tc, ExitStack() as ctx:
        in_p = ctx.enter_context(tc.tile_pool(name="inp", bufs=1))
        e_p = ctx.enter_context(tc.tile_pool(name="e", bufs=6))
        w_p = ctx.enter_context(tc.tile_pool(name="w", bufs=4))
        t_p = ctx.enter_context(tc.tile_pool(name="tree", bufs=2))
        u_p = ctx.enter_context(tc.tile_pool(name="uq", bufs=2))
        r_p = ctx.enter_context(tc.tile_pool(name="rq", bufs=2))
        st_p = ctx.enter_context(tc.tile_pool(name="stage", bufs=1))
        ps_s = ctx.enter_context(tc.tile_pool(name="ps_s", bufs=2, space="PSUM"))
        ps_o = ctx.enter_context(tc.tile_pool(name="ps_o", bufs=1, space="PSUM"))

        # kt/vv are laid out k-tile-major on the host and DMA'd in per-tile
        # chunks interleaved kt/vv, so tile 0's operands land ~7us in and the
        # loop never waits on later chunks.
        qt = in_p.tile([128, NPAIR * QBLK], f16)
        kt = in_p.tile([128, NKT * NPAIR * KT], f16)
        vv = in_p.tile([128, NKT * B * D], f16)
        CH = NPAIR * KT  # 512 columns per k-tile chunk (for both kt and vv)

        def dma_col(dst, src, c0, c1):
            nc.sync.dma_start(out=dst[:, c0:c1], in_=src[:, c0:c1])

        # Issue order: operands of score pack (t=0, j=0) first, then the
        # rest of tile 0, then per-tile chunks so the loop never waits.
        dma_col(qt, qt_d, 0, QBLK)
        dma_col(kt, kt_d, 0, CH)
        for j in range(1, NPAIR):
            dma_col(qt, qt_d, j * QBLK, (j + 1) * QBLK)
        dma_col(vv, vv_d, 0, CH)
        for t in range(1, NKT):
            dma_col(kt, kt_d, t * CH, (t + 1) * CH)
            dma_col(vv, vv_d, t * CH, (t + 1) * CH)

        # Persistent output accumulators: bank j holds batches 2j (parts
        # 0:64) and 2j+1 (parts 64:128), accumulated over all 32 k-tiles.
        oacc = [
            ps_o.tile([128, QBLK], f32, tag=f"oacc{j}", name=f"oacc{j}")
            for j in range(NPAIR)
        ]

        # AV matmuls pending issue; drained between score packs so PE always
        # services the (ACT-feeding) score matmuls promptly instead of
        # running long AV bursts that starve ScalarE. Interleaving AV MMs
        # of adjacent k-tiles is safe: psum accumulate-adds commute.
        av_pending = []

        def drain_av(n):
            for _ in range(min(n, len(av_pending))):
                av_pending.pop(0)()

        def emit_scores_exp(t):
            # scores + exp; all four packs' exps land in ONE contiguous
            # [128, 4096] fp16 e-tile so the tree adds and the normalize
            # multiply below run as few wide DVE ops.
            e = e_p.tile([128, 4 * 2 * QBLK], f16, tag="e", name=f"e{t}")
            for j in range(NPAIR):
                sc = ps_s.tile([128, 2 * QBLK], f32, tag="sc", name=f"sc{t}_{j}")
                for m in range(2):  # m=0 -> b=2j (rows 0:64), m=1 -> b=2j+1
                    rb = m * 64
                    nc.tensor.matmul(
                        out=sc[:, m * QBLK : (m + 1) * QBLK],
                        lhsT=kt[rb : rb + 64, t * CH + j * KT : t * CH + (j + 1) * KT],
                        rhs=qt[rb : rb + 64, j * QBLK : (j + 1) * QBLK],
                        start=True,
                        stop=True,
                        tile_position=(rb, 0),
                    )
                # E = exp(scores / sqrt(D)); scores*0.125 in [-6, 6] so no
                # max-subtraction is needed and fp16 range is safe.
                nc.scalar.activation(
                    e[:, j * 2 * QBLK : (j + 1) * 2 * QBLK], sc[:], Exp, scale=0.125
                )
                drain_av(2)
            return e

        def emit_tree(t, e, uq):
            # First two levels of the 8-batch sum on DVE (fp16 2x mode);
            # the final level is fused into the reciprocal op.
            W2 = 2 * 2 * QBLK  # 2048
            tt = t_p.tile([128, W2], f16, tag="t", name=f"t{t}")
            nc.vector.tensor_add(tt[:], e[:, :W2], e[:, W2:])
            nc.vector.tensor_add(
                uq[:, uq_pos(t) * 2 * QBLK : (uq_pos(t) + 1) * 2 * QBLK],
                tt[:, : 2 * QBLK],
                tt[:, 2 * QBLK :],
            )

        # group bookkeeping
        tile_group = {}
        for gi, (g0, gn) in enumerate(GROUPS):
            for u in range(g0, g0 + gn):
                tile_group[u] = (gi, g0, gn)

        def uq_pos(t):
            return t - tile_group[t][1]

        def uq_len(t):
            return tile_group[t][2]

        def emit_recip(gi, g0, gn, uq):
            # r = 1/(u0 + u1) for all gn tiles of the group in ONE custom
            # DVE op: [128, gn, 512] strided views of the group's U buffer.
            rq = r_p.tile([128, gn * QBLK], f16, tag="rq", name=f"rq{gi}")
            uqv = uq[:, : gn * 2 * QBLK].rearrange(
                "p (g c) -> p g c", g=gn
            )
            nc.vector._custom_dve(
                zsum_recip,
                out=rq[:].rearrange("p (g q) -> p g q", g=gn),
                in0=uqv[:, :, :QBLK],
                in1=uqv[:, :, QBLK:],
                s0=_RECIP_C0,
                s1=_RECIP_C1,
            )
            return rq

        def emit_mult(t, e, rq, g):
            # W_b = E_b * r, one fp16 2x-mode op for the whole k-tile with r
            # broadcast over the 8 (pack, half) chunks via a stride-0 AP.
            w = w_p.tile([128, 8 * QBLK], f16, tag="w", name=f"w{t}")
            r = rq[:, g * QBLK : (g + 1) * QBLK]
            nc.vector.tensor_mul(
                w[:].rearrange("p (a q) -> p a q", a=8),
                e[:].rearrange("p (a q) -> p a q", a=8),
                r.rearrange("p (a q) -> p a q", a=1).to_broadcast((128, 8, QBLK)),
            )
            return w

        def emit_av(t, w):
            # outT_b[d,q] += V_b[t]^T-form matmul, queued for interleaved
            # issue (see drain_av). Reverse order so the first-issued AV's
            # wait (on the mult's DVE tick) covers the others.
            def mk(b):
                j, m = b // 2, b % 2
                rb = m * 64

                def go():
                    nc.tensor.matmul(
                        out=oacc[j][rb : rb + 64, :],
                        lhsT=vv[:, t * CH + b * D : t * CH + (b + 1) * D],
                        rhs=w[:, b * QBLK : (b + 1) * QBLK],
                        start=(t == 0),
                        stop=(t == NKT - 1),
                        tile_position=(0, rb),
                        skip_group_check=True,
                    )

                return go

            for b in reversed(range(B)):
                av_pending.append(mk(b))

        # Software pipeline: front end per tile t = scores+exp (PE+ACT) and
        # the two tree adds (DVE). Back end per GROUP, triggered one tile
        # after the group's last tree add: the fused reciprocal, then the
        # normalize multiplies + AV matmuls for every tile of the group.
        # All back-end deps are same-engine (DVE) or >=1 tile old, so no
        # engine's in-order queue ever head-of-line blocks on fresh data.
        trigger = {g0 + gn: (gi, g0, gn) for gi, (g0, gn) in enumerate(GROUPS)}
        uq_of_group = {}
        e_of_tile = {}

        for t in range(NKT + 1):
            if t < NKT:
                gi = tile_group[t][0]
                if uq_pos(t) == 0:
                    uq_of_group[gi] = u_p.tile(
                        [128, uq_len(t) * 2 * QBLK], f16, tag="uq", name=f"uq{gi}"
                    )
                e = emit_scores_exp(t)
                e_of_tile[t] = e
                emit_tree(t, e, uq_of_group[gi])
            if t in trigger:
                gi, g0, gn = trigger[t]
                rq = emit_recip(gi, g0, gn, uq_of_group.pop(gi))
                for u in range(g0, g0 + gn):
                    w = emit_mult(u, e_of_tile.pop(u), rq, u - g0)
                    emit_av(u, w)
        drain_av(len(av_pending))

        # One staging tile + one output DMA (4 separate dma_starts each pay
        # ~2us of setup/completion latency in the kernel tail).
        st = st_p.tile([128, NPAIR * QBLK], f32, tag="st")
        for j in range(NPAIR):
            nc.vector.tensor_copy(
                out=st[:, j * QBLK : (j + 1) * QBLK], in_=oacc[j][:]
            )
        nc.sync.dma_start(
            out=out_d[:].rearrange("j p q -> p j q"),
            in_=st[:].rearrange("p (j q) -> p j q", j=NPAIR),
        )

    return nc


def _get_nc():
    if "nc" not in _cache:
        nc = _build_nc()
        if not nc.is_finalized():
            # Runs Bacc.compile() legalization (wait splitting, reg alloc).
            nc.finalize()
        _cache["nc"] = nc
    return _cache["nc"]


def _host_prep(queries, keys, values):
    """Cast to fp16 and pre-arrange into the SBUF layouts (see _build_nc)."""
    k16 = np.asarray(keys, dtype=np.float16)
    v16 = np.asarray(values, dtype=np.float16)
    q16 = np.asarray(queries, dtype=np.float16)

    # kt[(b%2)*64+d, t*512 + (b//2)*128 + kk] = K[b, t*128+kk, d] (k-tile major)
    kt = np.ascontiguousarray(
        k16.reshape(NPAIR, 2, NKT, KT, D)
        .transpose(1, 4, 2, 0, 3)
        .reshape(128, NKT * NPAIR * KT)
    )
    # vv[p, t*512 + b*64 + d] = V[b, t*128+p, d] (k-tile major)
    vv = np.ascontiguousarray(
        v16.reshape(B, NKT, KT, D).transpose(2, 1, 0, 3).reshape(128, NKT * B * D)
    )

    qts = []
    for c in range(NCORES):
        qc = q16[:, c * QBLK : (c + 1) * QBLK, :]  # [8, 512, 64]
        qt = np.ascontiguousarray(
            qc.transpose(0, 2, 1).reshape(NPAIR, 128, QBLK).transpose(1, 0, 2).reshape(128, NPAIR * QBLK)
        )
        qts.append(qt)
    return qts, kt, vv


def kernel(queries, keys, values):
    global LAST_RESULT
    from concourse.bass_utils import run_bass_kernel_spmd

    queries = np.asarray(queries, dtype=np.float32)
    keys = np.asarray(keys, dtype=np.float32)
    values = np.asarray(values, dtype=np.float32)

    nc = _get_nc()
    qts, kt, vv = _host_prep(queries, keys, values)
    in_maps = [{"qt": qts[c], "kt": kt, "vv": vv} for c in range(NCORES)]

    res = run_bass_kernel_spmd(
        nc,
        in_maps,
        list(range(NCORES)),
        trace=TRACE,
        **TRACE_KWARGS,
    )
    LAST_RESULT = res

    out = np.empty((B, S, D), dtype=np.float32)
    for c in range(NCORES):
        o = res.results[c]["out"]  # [4, 128, 512] = [j, (b%2)*64+d, q]
        out[:, c * QBLK : (c + 1) * QBLK, :] = (
            o.reshape(B, D, QBLK).transpose(0, 2, 1)
        )
    return out
